# revision 1
# baseline (speedup 1.0000x reference)
"""DiT block kernel for 8 Trainium2 NeuronCores (Bass/Tile).

Sharding: sequence dim L=4096 split 8 ways (512 query rows per core).
Each core computes K/V for the full sequence (replicated; no cross-core
collective) but only its own 512 queries through attention, the
out-projection, and the FFN. Inputs are rotated host-side so every
core's local rows sit at positions [0, 512) -> one SPMD program.

Precision: fp8e4 (e4m3) DoubleRow matmuls for all projections, the FFN
and the attention scores (2x TensorE throughput); bf16 for attn @ V;
fp32 accumulation in PSUM; layernorm stats and residuals fp32.
AdaLN modulation is fused into the bf16->fp8 convert of the transposed
LN output. Softmax exp is split between ScalarE (native Exp) and
VectorE (Schraudolph int16-bitcast approximation).
"""

import sys

sys.path.insert(0, "/opt/trn_rl_repo")

from contextlib import ExitStack

import numpy as np
import ml_dtypes

import concourse.bass as bass
import concourse.bacc as bacc
import concourse.tile as tile
import concourse.mybir as mybir
from concourse.bass_utils import run_bass_kernel_spmd

F32 = mybir.dt.float32
BF16 = mybir.dt.bfloat16
FP8 = mybir.dt.float8e4
FP8E5 = mybir.dt.float8e5
I8 = mybir.dt.int8
I16 = mybir.dt.int16
AF = mybir.ActivationFunctionType
OP = mybir.AluOpType
DR = mybir.MatmulPerfMode.DoubleRow

L, D, H, HD, DM = 4096, 768, 12, 64, 3072
NCORES = 8
LQ = L // NCORES  # 512 local query rows
P = 128
EPS = 1e-5
NLC = L // 512  # 8 l-chunks of 512
NKC = L // P  # 32 k-chunks of 128
NQC = LQ // P  # 4 local q-chunks of 128
NDC = D // P  # 6 chunks of the model dim
NHP = H // 2  # 6 head pairs
NMC = DM // P  # 24 chunks of the FFN hidden dim

# host-side power-of-two scales applied to fp8 weights (exact to undo)
SW_QKV = 64.0
SW_AO = 16.0
SW_F1 = 16.0
SW_F2 = 32.0

# Schraudolph exp -> fp8e5 bits: i8 = round(x*2^2/ln2 + (15*2^2 - C))
SCH_A = 5.770780 * 0.125  # includes the hd^-0.5 = 1/8 score scale
SCH_B = 59.77


def _declare_params(nc):
    dp = nc.declare_dram_parameter
    t = {}
    t["x"] = dp("x", [LQ, D], F32, isOutput=False)
    t["x_bf"] = dp("x_bf", [L, D], BF16, isOutput=False)
    t["cond_t"] = dp("cond_t", [P, NDC], F32, isOutput=False)
    t["w_adaln1"] = dp("w_adaln1", [D, 3 * D], BF16, isOutput=False)
    t["w_adaln2"] = dp("w_adaln2", [D, 3 * D], BF16, isOutput=False)
    t["b_adaln1_col"] = dp("b_adaln1_col", [P, 12], F32, isOutput=False)
    t["b_adaln2_col"] = dp("b_adaln2_col", [P, 12], F32, isOutput=False)
    t["b_adaln1_gate"] = dp("b_adaln1_gate", [1, D], F32, isOutput=False)
    t["b_adaln2_gate"] = dp("b_adaln2_gate", [1, D], F32, isOutput=False)
    t["w_qkv8"] = dp("w_qkv8", [D, 3 * D], FP8, isOutput=False)  # * SW_QKV
    t["b_qkv_col"] = dp("b_qkv_col", [P, 18], F32, isOutput=False)
    t["w_ao8"] = dp("w_ao8", [D, D], FP8, isOutput=False)  # * SW_AO
    t["b_attn_b"] = dp("b_attn_b", [P, D], F32, isOutput=False)  # b_attn + bv@Wao
    t["w_ffn18"] = dp("w_ffn18", [D, DM], FP8, isOutput=False)  # * SW_F1
    t["b_ffn1_col"] = dp("b_ffn1_col", [P, NMC], F32, isOutput=False)
    t["w_f28"] = dp("w_f28", [DM, D], FP8, isOutput=False)  # * SW_F2
    t["b_ffn2_b"] = dp("b_ffn2_b", [P, D], F32, isOutput=False)
    t["out"] = dp("out", [LQ, D], F32, isOutput=True)
    return t


def _build_body(nc, tc, ctx, t):
    mm = nc.tensor.matmul
    dma = nc.sync.dma_start
    dmat = nc.sync.dma_start_transpose
    v = nc.vector
    act = nc.scalar.activation

    const = ctx.enter_context(tc.tile_pool(name="const", bufs=1))
    eps_t = const.tile([P, 1], F32)
    v.memset(eps_t, EPS)

    # ---------------- phase A: cond path (SiLU + AdaLN projections) -------
    adaln = ctx.enter_context(tc.tile_pool(name="adaln", bufs=1))
    sc_bf = adaln.tile([P, NDC], BF16)
    sh1_col = adaln.tile([P, NDC], F32)
    sp1_col = adaln.tile([P, NDC], F32)
    sh2_col = adaln.tile([P, NDC], F32)
    sp2_col = adaln.tile([P, NDC], F32)
    g1s_b = adaln.tile([P, D], F32)  # g1, broadcast
    g2s_b = adaln.tile([P, D], F32)  # g2, broadcast
    xb_bias = adaln.tile([P, D], F32)  # g1 * b_attn_eff
    x2b_bias = adaln.tile([P, D], F32)  # g2 * b_ffn2
    wa1g = adaln.tile([P, NDC, D], BF16)  # gate columns of w_adaln1
    wa2g = adaln.tile([P, NDC, D], BF16)
    b1g_k = adaln.tile([1, D], F32)
    b2g_k = adaln.tile([1, D], F32)

    with ExitStack() as phA:
        pool = phA.enter_context(tc.tile_pool(name="phA", bufs=1))
        psA1 = phA.enter_context(tc.tile_pool(name="psA1", bufs=4, space="PSUM"))
        psA2 = phA.enter_context(tc.tile_pool(name="psA2", bufs=2, space="PSUM"))

        cond_sb = pool.tile([P, NDC], F32)
        dma(out=cond_sb[:], in_=t["cond_t"][:])
        sc_f = pool.tile([P, NDC], F32)
        act(sc_f[:], cond_sb[:], AF.Silu)
        v.tensor_copy(sc_bf[:], sc_f[:])

        wa1 = pool.tile([P, NDC, 2 * D], BF16)
        wa2 = pool.tile([P, NDC, 2 * D], BF16)
        wa1_r = t["w_adaln1"].rearrange("(c p) m -> p c m", p=P)
        wa2_r = t["w_adaln2"].rearrange("(c p) m -> p c m", p=P)
        for dc in range(NDC):
            dma(out=wa1[:, dc, :], in_=wa1_r[:, dc, 0 : 2 * D])
        for dc in range(NDC):
            dma(out=wa2[:, dc, :], in_=wa2_r[:, dc, 0 : 2 * D])
        for dc in range(NDC):
            dma(out=wa1g[:, dc, :], in_=wa1_r[:, dc, 2 * D :])
        for dc in range(NDC):
            dma(out=wa2g[:, dc, :], in_=wa2_r[:, dc, 2 * D :])
        b1c = pool.tile([P, 12], F32)
        dma(out=b1c[:], in_=t["b_adaln1_col"][:])
        b2c = pool.tile([P, 12], F32)
        dma(out=b2c[:], in_=t["b_adaln2_col"][:])
        b1g = b1g_k
        dma(out=b1g[:], in_=t["b_adaln1_gate"][:])
        b2g = b2g_k
        dma(out=b2g[:], in_=t["b_adaln2_gate"][:])

        for r, (wa, bc, sh_col, sp_col) in enumerate(
            [
                (wa1, b1c, sh1_col, sp1_col),
                (wa2, b2c, sh2_col, sp2_col),
            ]
        ):
            acol = pool.tile([P, 12], F32, name=f"acol{r}")
            for m in range(12):
                ps = psA1.tile([P, 1], F32)
                for dc in range(NDC):
                    mm(
                        ps[:],
                        wa[:, dc, m * P : (m + 1) * P],
                        sc_bf[:, dc : dc + 1],
                        start=(dc == 0),
                        stop=(dc == NDC - 1),
                    )
                v.tensor_add(acol[:, m : m + 1], ps[:], bc[:, m : m + 1])
            v.tensor_copy(sh_col[:], acol[:, 0:6])
            v.tensor_scalar_add(sp_col[:], acol[:, 6:12], 1.0)


    # gates (needed only from phase D onward): computed after phase B setup
    def emit_gates(gpool, gps):
        for r, (wa, bg, gs_b) in enumerate(
            [(wa1g, b1g_k, g1s_b), (wa2g, b2g_k, g2s_b)]
        ):
            g_row = gpool.tile([1, D], F32, name=f"grow{r}")
            for j, (n0, n1) in enumerate([(0, 512), (512, 768)]):
                ps = gps.tile([1, n1 - n0], F32, tag="psg")
                for dc in range(NDC):
                    mm(
                        ps[:],
                        sc_bf[:, dc : dc + 1],
                        wa[:, dc, :][:, n0:n1],
                        start=(dc == 0),
                        stop=(dc == NDC - 1),
                    )
                v.tensor_add(g_row[:, n0:n1], ps[:], bg[:, n0:n1])
            nc.gpsimd.partition_broadcast(gs_b[:], g_row[:])

    # ---------------- phase B: LN1 -> xn1T (fp8, modulated) + V/Q ---------
    big = ctx.enter_context(tc.tile_pool(name="big", bufs=1))
    x_loc = big.tile([P, NQC, D], F32)  # local rows for the residual
    x2_loc = [big.tile([P, D], F32, name=f"x2_loc{q}") for q in range(NQC)]
    catT8 = big.tile([P, NDC, LQ], FP8)  # attention output, transposed
    xn2T8 = big.tile([P, NDC, LQ], FP8)
    s_attn = ctx.enter_context(ExitStack())
    attn_pool = s_attn.enter_context(tc.tile_pool(name="attn", bufs=1))
    xn1T8 = [
        attn_pool.tile([P, NDC, 512], FP8, name=f"xn1T8_{i}") for i in range(NLC)
    ]
    v_all = attn_pool.tile([P, NKC, H * (HD + 4)], FP8)  # V + ones + pad (stride %16)
    qT_all = attn_pool.tile([P, NHP, LQ], BF16)
    w8qkv = attn_pool.tile([P, NDC, 3 * D], FP8)
    bq_col = attn_pool.tile([P, 18], F32)

    dma(out=w8qkv[:], in_=t["w_qkv8"].rearrange("(c p) m -> p c m", p=P))
    dma(out=bq_col[:], in_=t["b_qkv_col"][:])
    dma(out=x_loc[:], in_=t["x"].rearrange("(n p) d -> p n d", p=P))
    v.memset(
        v_all.rearrange("p k (h e) -> p k h e", e=HD + 4)[:, :, :, HD : HD + 1], 1.0
    )
    v.memset(
        v_all.rearrange("p k (h e) -> p k h e", e=HD + 4)[:, :, :, HD + 1 :], 0.0
    )

    with ExitStack() as phB:
        xload = phB.enter_context(tc.tile_pool(name="xload", bufs=10))
        spool = phB.enter_context(tc.tile_pool(name="spool", bufs=10))
        nxpool = phB.enter_context(tc.tile_pool(name="nxpool", bufs=10))
        tpool = phB.enter_context(tc.tile_pool(name="tpool", bufs=3))
        psB1 = phB.enter_context(tc.tile_pool(name="psB1", bufs=2, space="PSUM"))
        psB2 = phB.enter_context(tc.tile_pool(name="psB2", bufs=2, space="PSUM"))

        v4 = v_all.rearrange("p k (h e) -> p k h e", e=HD + 4)
        xn1T_bfs = [None] * NLC
        # LN1 over the full sequence, transposed via DMA-xbar, modulated
        # into fp8 during the convert. Processed in blocks of 8 chunks so
        # each engine sees long runs of independent ops (FIFO queues).
        x_r = t["x_bf"].rearrange("(n p) d -> n p d", p=P)
        BLK = 8
        for b in range(NKC // BLK):
            xts, mvs, sqs, rstds, nxs = [], [], [], [], []
            for ii in range(BLK):
                i = b * BLK + ii
                xt = xload.tile([P, D], BF16, tag="xt")
                dma(out=xt[:], in_=x_r[i])
                xts.append(xt)
            for ii in range(BLK):
                stats = spool.tile([P, 2, 6], F32, tag="stats")
                for g in range(2):
                    v.bn_stats(stats[:, g, :], xts[ii][:, g * 384 : (g + 1) * 384])
                mv = spool.tile([P, 2], F32, tag="mv")
                v.bn_aggr(mv[:], stats[:])
                mvs.append(mv)
            for ii in range(BLK):
                sq = spool.tile([P, 1], F32, tag="sq")
                act(sq[:], mvs[ii][:, 1:2], AF.Sqrt, bias=eps_t[:, 0:1])
                sqs.append(sq)
            for ii in range(BLK):
                rstd = spool.tile([P, 1], F32, tag="rstd")
                v.reciprocal_approx_fast(rstd[:], sqs[ii][:])
                rstds.append(rstd)
            for ii in range(BLK):
                i = b * BLK + ii
                nx = nxpool.tile([P, D], BF16, tag="nx")
                v.tensor_scalar(
                    nx[:], xts[ii][:], mvs[ii][:, 0:1], rstds[ii][:, 0:1],
                    op0=OP.subtract, op1=OP.mult,
                )
                nxs.append(nx)
            for ii in range(BLK):
                i = b * BLK + ii
                lc = i // 4
                if i % 4 == 0:
                    xn1T_bfs[lc] = tpool.tile([P, NDC, 512], BF16, tag="xnbf", name=f"xnbf{lc}")
                dmat(out=xn1T_bfs[lc][:, :, (i % 4) * P : (i % 4 + 1) * P], in_=nxs[ii][:])
            for lc in (b * 2, b * 2 + 1):
                for dc in range(NDC):
                    v.tensor_scalar(
                        xn1T8[lc][:, dc, :],
                        xn1T_bfs[lc][:, dc, :],
                        sp1_col[:, dc : dc + 1],
                        sh1_col[:, dc : dc + 1],
                        op0=OP.mult,
                        op1=OP.add,
                    )
                # V for this strip (fp8 DoubleRow); bias folded into b_attn
                for kc in range(lc * 4, lc * 4 + 4):
                    ps_v = psB2.tile([P, D], F32, tag="psv")
                    for dc2 in range(NDC // 2):
                        lhs = xn1T8[lc][:, 2 * dc2 : 2 * dc2 + 2, (kc % 4) * P : (kc % 4 + 1) * P]
                        mm(ps_v[:, 0:512], lhs, w8qkv[:, 2 * dc2 : 2 * dc2 + 2, 2 * D : 2 * D + 512],
                           start=(dc2 == 0), stop=(dc2 == NDC // 2 - 1), perf_mode=DR)
                        mm(ps_v[:, 512:768], lhs, w8qkv[:, 2 * dc2 : 2 * dc2 + 2, 2 * D + 512 : 3 * D],
                           start=(dc2 == 0), stop=(dc2 == NDC // 2 - 1), perf_mode=DR)
                    act(
                        v4[:, kc, :, 0:HD],
                        ps_v.rearrange("p (h e) -> p h e", e=HD),
                        AF.Copy,
                        scale=1.0 / SW_QKV,
                    )
                if lc == 0:
                    # Q^T bf16 (local rows only)
                    for hp in range(NHP):
                        ps_q = psB1.tile([P, 512], F32, tag="mm512")
                        for dc2 in range(NDC // 2):
                            mm(
                                ps_q[:],
                                w8qkv[:, 2 * dc2 : 2 * dc2 + 2, hp * P : (hp + 1) * P],
                                xn1T8[0][:, 2 * dc2 : 2 * dc2 + 2, :],
                                start=(dc2 == 0),
                                stop=(dc2 == NDC // 2 - 1),
                                perf_mode=DR,
                            )
                        v.tensor_scalar(
                            qT_all[:, hp, :], ps_q[:], 1.0 / SW_QKV, bq_col[:, hp : hp + 1],
                            op0=OP.mult, op1=OP.add,
                        )

    # ------- phase C: merged K-projection + attention pipeline -------------
    with ExitStack() as phC:
        kv_pool = phC.enter_context(tc.tile_pool(name="kvp", bufs=2))
        pt_pool = phC.enter_context(tc.tile_pool(name="ptp", bufs=4))
        tiny = phC.enter_context(tc.tile_pool(name="tiny", bufs=2))
        rzb_pool = phC.enter_context(tc.tile_pool(name="rzb", bufs=2))
        psS = phC.enter_context(tc.tile_pool(name="psS", bufs=2, space="PSUM"))
        psO = phC.enter_context(tc.tile_pool(name="psO", bufs=2, space="PSUM"))
        psK = phC.enter_context(tc.tile_pool(name="psK", bufs=2, space="PSUM"))

        def emit_K(hp, kT):
            for lc in range(NLC):
                ps_k = psK.tile([P, 512], F32)
                for dc2 in range(NDC // 2):
                    mm(
                        ps_k[:],
                        w8qkv[:, 2 * dc2 : 2 * dc2 + 2, D + hp * P : D + (hp + 1) * P],
                        xn1T8[lc][:, 2 * dc2 : 2 * dc2 + 2, :],
                        start=(dc2 == 0),
                        stop=(dc2 == NDC // 2 - 1),
                        perf_mode=DR,
                    )
                act(
                    kT[:, lc * 512 : (lc + 1) * 512],
                    ps_k[:],
                    AF.Identity,
                    bias=bq_col[:, 6 + hp : 7 + hp],
                    scale=1.0 / SW_QKV,
                )

        kT_cur = kv_pool.tile([P, L], BF16, tag="kT", name="kT0")
        emit_K(0, kT_cur)
        for hp in range(NHP):
            kT = kT_cur
            kT_nxt = None
            if hp + 1 < NHP:
                kT_nxt = kv_pool.tile([P, L], BF16, tag="kT", name=f"kT{hp + 1}")
            for dlt in range(2):
                h, off = 2 * hp + dlt, dlt * HD
                ps_o = psO.tile([HD + 4, 512], F32)
                pending = None
                for kc2 in range(NKC // 2):
                    ps_s = psS.tile([P, 1024], F32)
                    for j in range(2):
                        kc = 2 * kc2 + j
                        mm(
                            ps_s[:, j * 512 : (j + 1) * 512],
                            kT[off : off + HD, kc * P : (kc + 1) * P],
                            qT_all[off : off + HD, hp, :],
                            start=True,
                            stop=True,
                        )
                    ptile = pt_pool.tile([P, 1024], FP8E5)
                    if kc2 % 5 in (1, 3):
                        # Schraudolph exp on VectorE: int8 bits of fp8e5 e^x
                        v.tensor_scalar(
                            ptile.bitcast(I8)[:], ps_s[:], SCH_A, SCH_B,
                            op0=OP.mult, op1=OP.add,
                        )
                    else:
                        act(ptile[:], ps_s[:], AF.Exp, scale=0.125)
                    # AV (DoubleRow over kc pairs) for the PREVIOUS tile -
                    # keeps the PE fed while the current tile's exp runs.
                    if pending is not None:
                        pk2, ppt = pending
                        mm(
                            ps_o[:],
                            v_all[:, 2 * pk2 : 2 * pk2 + 2, h * (HD + 4) : (h + 1) * (HD + 4)],
                            ppt.rearrange("p (j n) -> p j n", j=2)[:],
                            start=(pk2 == 0),
                            stop=False,
                            perf_mode=DR,
                        )
                    pending = (kc2, ptile)
                    if dlt == 0 and kc2 == 7 and kT_nxt is not None:
                        # K for the next head pair, fed while exp pipelines
                        emit_K(hp + 1, kT_nxt)
                pk2, ppt = pending
                mm(
                    ps_o[:],
                    v_all[:, 2 * pk2 : 2 * pk2 + 2, h * (HD + 4) : (h + 1) * (HD + 4)],
                    ppt.rearrange("p (j n) -> p j n", j=2)[:],
                    start=False,
                    stop=True,
                    perf_mode=DR,
                )
                # normalize columns by the ones-row (softmax denominator)
                zrow = tiny.tile([1, 512], F32)
                v.tensor_copy(zrow[:], ps_o[HD : HD + 1, :])
                rz = tiny.tile([1, 512], F32, tag="rz")
                v.reciprocal_approx_fast(rz[:], zrow[:])
                rz_b = rzb_pool.tile([P, 512], F32)
                nc.gpsimd.partition_broadcast(rz_b[:], rz[:])
                v.tensor_tensor(
                    catT8[off : off + HD, hp, :],
                    ps_o[0:HD, :],
                    rz_b[0:HD, :],
                    op=OP.mult,
                )
            kT_cur = kT_nxt

    s_attn.close()  # free K/V/Q/xn1T space before the FFN weights land

    # -------- phase D: out-projection, residual, LN2 (per-qc fused) ------
    with ExitStack() as phD:
        pool = phD.enter_context(tc.tile_pool(name="phD", bufs=2))
        spool = phD.enter_context(tc.tile_pool(name="spoolE", bufs=4))
        nxpool = phD.enter_context(tc.tile_pool(name="nxE", bufs=4))
        tpool2 = phD.enter_context(tc.tile_pool(name="tpool2", bufs=1))
        psD1 = phD.enter_context(tc.tile_pool(name="psD1", bufs=2, space="PSUM"))
        psD2 = phD.enter_context(tc.tile_pool(name="psD2", bufs=2, space="PSUM"))

        gps = phD.enter_context(tc.tile_pool(name="gps", bufs=2, space="PSUM"))
        emit_gates(pool, gps)
        w8ao = pool.tile([P, NDC, D], FP8, name="w8ao")
        dma(out=w8ao[:], in_=t["w_ao8"].rearrange("(c p) m -> p c m", p=P))
        ba_sb = pool.tile([P, D], F32, name="ba_sb")
        dma(out=ba_sb[:], in_=t["b_attn_b"][:])
        bf2_sb = pool.tile([P, D], F32, name="bf2_sb")
        dma(out=bf2_sb[:], in_=t["b_ffn2_b"][:])
        v.tensor_tensor(xb_bias[:], ba_sb[:], g1s_b[:], op=OP.mult)
        v.tensor_tensor(x2b_bias[:], bf2_sb[:], g2s_b[:], op=OP.mult)
        # xbl = x + g1*b_attn_eff, the per-qc residual base
        xbl = [pool.tile([P, D], F32, name=f"xbl{q}") for q in range(NQC)]
        for q in range(NQC):
            v.tensor_add(xbl[q][:], x_loc[:, q, :], xb_bias[:])

        xn2T_bf = tpool2.tile([P, NDC, LQ], BF16)
        gts, mvs2, sqs2, rstds2 = [], [], [], []
        for qc in range(NQC):
            ps1 = psD1.tile([P, 512], F32)
            ps2 = psD2.tile([P, 256], F32)
            for cc2 in range(NDC // 2):
                lhs = catT8[:, 2 * cc2 : 2 * cc2 + 2, qc * P : (qc + 1) * P]
                mm(ps1[:], lhs, w8ao[:, 2 * cc2 : 2 * cc2 + 2, 0:512],
                   start=(cc2 == 0), stop=(cc2 == NDC // 2 - 1), perf_mode=DR)
                mm(ps2[:], lhs, w8ao[:, 2 * cc2 : 2 * cc2 + 2, 512:768],
                   start=(cc2 == 0), stop=(cc2 == NDC // 2 - 1), perf_mode=DR)
            # x2 = x + g1*(psum / SW_AO) + g1*b
            gt = pool.tile([P, D], F32, tag="gt", name=f"gt{qc}")
            v.scalar_tensor_tensor(
                gt[:, 0:512], ps1[:], 1.0 / SW_AO, g1s_b[:, 0:512],
                op0=OP.mult, op1=OP.mult,
            )
            v.scalar_tensor_tensor(
                gt[:, 512:768], ps2[:], 1.0 / SW_AO, g1s_b[:, 512:768],
                op0=OP.mult, op1=OP.mult,
            )
            v.tensor_add(x2_loc[qc][:], gt[:], xbl[qc][:])
        for qc in range(NQC):
            stats = spool.tile([P, 2, 6], F32, tag="st2")
            for g in range(2):
                v.bn_stats(stats[:, g, :], x2_loc[qc][:, g * 384 : (g + 1) * 384])
            mv = spool.tile([P, 2], F32, tag="mv2", name=f"mv2_{qc}")
            v.bn_aggr(mv[:], stats[:])
            mvs2.append(mv)
        for qc in range(NQC):
            sq = spool.tile([P, 1], F32, tag="sq2", name=f"sq2_{qc}")
            act(sq[:], mvs2[qc][:, 1:2], AF.Sqrt, bias=eps_t[:, 0:1])
            sqs2.append(sq)
        for qc in range(NQC):
            rstd = spool.tile([P, 1], F32, tag="rstd2", name=f"rstd2_{qc}")
            v.reciprocal_approx_fast(rstd[:], sqs2[qc][:])
            rstds2.append(rstd)
        for qc in range(NQC):
            nmr = spool.tile([P, 1], F32, tag="nmr2", name=f"nmr2_{qc}")
            v.scalar_tensor_tensor(
                nmr[:], mvs2[qc][:, 0:1], -1.0, rstds2[qc][:, 0:1],
                op0=OP.mult, op1=OP.mult,
            )
            nx = nxpool.tile([P, D], BF16, tag="nx2", name=f"nx2_{qc}")
            act(nx[:], x2_loc[qc][:], AF.Identity, bias=nmr[:, 0:1], scale=rstds2[qc][:, 0:1])
            dmat(out=xn2T_bf[:, :, qc * P : (qc + 1) * P], in_=nx[:])
        for dc in range(NDC):
            act(
                xn2T8[:, dc, :],
                xn2T_bf[:, dc, :],
                AF.Identity,
                bias=sh2_col[:, dc : dc + 1],
                scale=sp2_col[:, dc : dc + 1],
            )

    # ---------------- phase F: FFN + gate + residual -> output -------------
    with ExitStack() as phF:
        wpool = phF.enter_context(tc.tile_pool(name="wffn", bufs=1))
        hpool = phF.enter_context(tc.tile_pool(name="hT", bufs=1))
        pool = phF.enter_context(tc.tile_pool(name="phF", bufs=2))
        psF1 = phF.enter_context(tc.tile_pool(name="psF1", bufs=3, space="PSUM"))
        psF2 = phF.enter_context(tc.tile_pool(name="psF2", bufs=2, space="PSUM"))

        w8f1 = wpool.tile([P, NDC, DM], FP8)
        dma(out=w8f1[:], in_=t["w_ffn18"].rearrange("(c p) m -> p c m", p=P))
        bf1_col = wpool.tile([P, NMC], F32)
        dma(out=bf1_col[:], in_=t["b_ffn1_col"][:])
        w8f2 = wpool.tile([P, NMC, D], FP8)
        dma(out=w8f2[:], in_=t["w_f28"].rearrange("(c p) m -> p c m", p=P))
        # x2 with the gated ffn2 bias folded in (per-qc residual base)
        for q in range(NQC):
            v.tensor_add(x2_loc[q][:], x2_loc[q][:], x2b_bias[:])

        hT8 = hpool.tile([P, NMC, LQ], FP8)
        for mc in range(NMC):
            ps_h = psF1.tile([P, 512], F32, tag="mm512")
            for dc2 in range(NDC // 2):
                mm(
                    ps_h[:],
                    w8f1[:, 2 * dc2 : 2 * dc2 + 2, mc * P : (mc + 1) * P],
                    xn2T8[:, 2 * dc2 : 2 * dc2 + 2, :],
                    start=(dc2 == 0),
                    stop=(dc2 == NDC // 2 - 1),
                    perf_mode=DR,
                )
            act(
                hT8[:, mc, :], ps_h[:], AF.Gelu,
                bias=bf1_col[:, mc : mc + 1], scale=1.0 / SW_F1,
            )

        out_r = t["out"].rearrange("(n p) d -> n p d", p=P)
        for qc in range(NQC):
            ps1 = psF1.tile([P, 512], F32, tag="mm512")
            ps2 = psF2.tile([P, 256], F32)
            for mc2 in range(NMC // 2):
                lhs = hT8[:, 2 * mc2 : 2 * mc2 + 2, qc * P : (qc + 1) * P]
                mm(ps1[:], lhs, w8f2[:, 2 * mc2 : 2 * mc2 + 2, 0:512],
                   start=(mc2 == 0), stop=(mc2 == NMC // 2 - 1), perf_mode=DR)
                mm(ps2[:], lhs, w8f2[:, 2 * mc2 : 2 * mc2 + 2, 512:768],
                   start=(mc2 == 0), stop=(mc2 == NMC // 2 - 1), perf_mode=DR)
            gt = pool.tile([P, D], F32, tag="gt")
            v.scalar_tensor_tensor(
                gt[:, 0:512], ps1[:], 1.0 / SW_F2, g2s_b[:, 0:512],
                op0=OP.mult, op1=OP.mult,
            )
            v.scalar_tensor_tensor(
                gt[:, 512:768], ps2[:], 1.0 / SW_F2, g2s_b[:, 512:768],
                op0=OP.mult, op1=OP.mult,
            )
            ot = pool.tile([P, D], F32)
            v.tensor_add(ot[:], gt[:], x2_loc[qc][:])
            dma(out=out_r[qc], in_=ot[:])


def build_nc():
    nc = bacc.Bacc(None, target_bir_lowering=False, debug=False)
    t = _declare_params(nc)
    with tile.TileContext(nc) as tc:
        with ExitStack() as ctx:
            _build_body(nc, tc, ctx, t)
    nc.compile()
    return nc


_cache = {}


def _prep_in_maps(inputs):
    E4 = ml_dtypes.float8_e4m3fn
    bf = lambda a: np.ascontiguousarray(np.asarray(a, np.float32)).astype(
        ml_dtypes.bfloat16
    )
    f32 = lambda a: np.ascontiguousarray(np.asarray(a, np.float32))
    q8 = lambda a, s: np.ascontiguousarray(
        (np.asarray(a, np.float32) * s).astype(E4)
    )
    x = f32(inputs["x"]).reshape(L, D)
    cond = f32(inputs["cond"]).reshape(D)
    b_adaln1 = f32(inputs["b_adaln1"]).reshape(3 * D)
    b_adaln2 = f32(inputs["b_adaln2"]).reshape(3 * D)
    b_qkv = f32(inputs["b_qkv"]).reshape(3 * D)
    w_ao = f32(inputs["w_attn_out"])
    # fold the V bias through the out-projection: b_attn_eff = b + bv @ Wao
    b_attn_eff = f32(inputs["b_attn_out"]).reshape(D) + b_qkv[2 * D :] @ w_ao
    common = {
        "cond_t": np.ascontiguousarray(cond.reshape(NDC, P).T),
        "w_adaln1": bf(inputs["w_adaln1"]),
        "w_adaln2": bf(inputs["w_adaln2"]),
        "b_adaln1_col": np.ascontiguousarray(b_adaln1[: 12 * P].reshape(12, P).T),
        "b_adaln2_col": np.ascontiguousarray(b_adaln2[: 12 * P].reshape(12, P).T),
        "b_adaln1_gate": np.ascontiguousarray(b_adaln1[2 * D :][None]),
        "b_adaln2_gate": np.ascontiguousarray(b_adaln2[2 * D :][None]),
        "w_qkv8": q8(inputs["w_qkv"], SW_QKV),
        "b_qkv_col": np.ascontiguousarray(b_qkv.reshape(18, P).T),
        "w_ao8": q8(w_ao, SW_AO),
        "b_attn_b": np.ascontiguousarray(np.broadcast_to(b_attn_eff, (P, D))),
        "w_ffn18": q8(inputs["w_ffn1"], SW_F1),
        "b_ffn1_col": np.ascontiguousarray(
            f32(inputs["b_ffn1"]).reshape(NMC, P).T
        ),
        "w_f28": q8(inputs["w_ffn2"], SW_F2),
        "b_ffn2_b": np.ascontiguousarray(
            np.broadcast_to(f32(inputs["b_ffn2"]).reshape(D), (P, D))
        ),
    }
    in_maps = []
    for c in range(NCORES):
        m = dict(common)
        xr = np.roll(x, -c * LQ, axis=0)
        m["x"] = np.ascontiguousarray(xr[:LQ])
        m["x_bf"] = np.ascontiguousarray(xr.astype(ml_dtypes.bfloat16))
        in_maps.append(m)
    return in_maps


def kernel(**inputs):
    if "nc" not in _cache:
        _cache["nc"] = build_nc()
    nc = _cache["nc"]
    in_maps = _prep_in_maps(inputs)
    res = run_bass_kernel_spmd(nc, in_maps, list(range(NCORES)))
    out = np.concatenate([res.results[c]["out"] for c in range(NCORES)], axis=0)
    return out.reshape(1, L, D).astype(np.float32)


if __name__ == "__main__":
    rng = np.random.default_rng(0)
    fake = {
        "x": rng.standard_normal((1, L, D), dtype=np.float32),
        "cond": rng.standard_normal((1, D), dtype=np.float32),
        "w_adaln1": rng.standard_normal((D, 3 * D), dtype=np.float32) * 0.02,
        "b_adaln1": np.zeros(3 * D, np.float32),
        "w_qkv": rng.standard_normal((D, 3 * D), dtype=np.float32) * D**-0.5,
        "b_qkv": np.zeros(3 * D, np.float32),
        "w_attn_out": rng.standard_normal((D, D), dtype=np.float32) * D**-0.5,
        "b_attn_out": np.zeros(D, np.float32),
        "w_adaln2": rng.standard_normal((D, 3 * D), dtype=np.float32) * 0.02,
        "b_adaln2": np.zeros(3 * D, np.float32),
        "w_ffn1": rng.standard_normal((D, DM), dtype=np.float32) * D**-0.5,
        "b_ffn1": np.zeros(DM, np.float32),
        "w_ffn2": rng.standard_normal((DM, D), dtype=np.float32) * DM**-0.5,
        "b_ffn2": np.zeros(D, np.float32),
    }
    out = kernel(**fake)
    print("out", out.shape, out.dtype, np.abs(out).max())



# revision 12
# speedup vs baseline: 1.0008x; 1.0008x over previous
"""DiT block kernel for 8 Trainium2 NeuronCores (Bass/Tile).

Sharding: sequence dim L=4096 split 8 ways (512 rows per core), with
cross-core AllGather collectives for the two replicated tensors:
  - the AdaLN projections (each core computes 576 of the 4608 output
    columns, then AllGather),
  - K (bf16) and V (fp8) for the full sequence (each core projects its
    own 512 rows, then AllGather; the gather-back DMA lays K/V into the
    attention-ready SBUF layouts for free).
Inputs are rotated host-side so every core's local rows sit at
positions [0, 512) -> one SPMD program.

Precision: fp8e4 (e4m3) DoubleRow matmuls for QKV/out-proj/FFN; bf16
scores with the two heads of a pair issued to disjoint PE row-groups
(64-row tiles -> concurrent); fp8 AV with DoubleRow over key-chunk
pairs; fp32 accumulation in PSUM; layernorm stats and residuals fp32.
Softmax exp is split across ScalarE (native Exp), VectorE and Pool
(Schraudolph int8-bitcast approximation) round-robin.
"""

import sys

sys.path.insert(0, "/opt/trn_rl_repo")

from contextlib import ExitStack

import numpy as np
import ml_dtypes

import concourse.bass as bass
import concourse.bacc as bacc
import concourse.tile as tile
import concourse.mybir as mybir
from concourse.bass_utils import run_bass_kernel_spmd

F32 = mybir.dt.float32
BF16 = mybir.dt.bfloat16
FP8 = mybir.dt.float8e4
FP8E5 = mybir.dt.float8e5
I8 = mybir.dt.int8
AF = mybir.ActivationFunctionType
OP = mybir.AluOpType
DR = mybir.MatmulPerfMode.DoubleRow

L, D, H, HD, DM = 4096, 768, 12, 64, 3072
NCORES = 8
LQ = L // NCORES  # 512 local rows
P = 128
EPS = 1e-5
NKC = L // P  # 32 k-chunks of 128
NQC = LQ // P  # 4 local q-chunks of 128
NDC = D // P  # 6 chunks of the model dim
NHP = H // 2  # 6 head pairs
NMC = DM // P  # 24 chunks of the FFN hidden dim
ASL = (3 * D * 2) // NCORES  # 576 adaln output cols per core
VE = HD + 4  # V row stride (64 dims + ones + pad)

# host-side power-of-two scales applied to fp8 weights (exact to undo)
SW_QKV = 64.0
SW_AO = 16.0
SW_F1 = 16.0
SW_F2 = 32.0
SW_CAT = 16.0  # ones-row = 1/SW_CAT so cat lands in fp8e4's sweet spot

# Schraudolph exp -> fp8e5 bits: i8 = round(x*2^2/ln2*0.125 + C)
SCH_A = 5.770780 * 0.125  # includes the hd^-0.5 = 1/8 score scale
SCH_B = 59.77

KBYTES = NHP * P * LQ * 2  # K region bytes in the KV AG buffer (bf16)
VBYTES = NQC * P * D  # V region bytes (fp8)


def _declare_params(nc):
    dp = nc.declare_dram_parameter
    t = {}
    t["x"] = dp("x", [LQ, D], F32, isOutput=False)
    t["cond_t"] = dp("cond_t", [P, NDC], F32, isOutput=False)
    t["wad_sl"] = dp("wad_sl", [P, NDC, ASL], BF16, isOutput=False)
    t["bad_sl"] = dp("bad_sl", [P, 5], F32, isOutput=False)
    t["w_qkv8"] = dp("w_qkv8", [D, 3 * D], FP8, isOutput=False)  # * SW_QKV
    t["b_q_col"] = dp("b_q_col", [P, NDC], F32, isOutput=False)
    t["w_ao8"] = dp("w_ao8", [D, D], FP8, isOutput=False)  # * SW_AO
    t["b_attn_b"] = dp("b_attn_b", [P, D], F32, isOutput=False)  # b_attn + bv@Wao
    t["w_ffn18"] = dp("w_ffn18", [D, DM], FP8, isOutput=False)  # * SW_F1
    t["b_ffn1_col"] = dp("b_ffn1_col", [P, NMC], F32, isOutput=False)
    t["w_f28"] = dp("w_f28", [DM, D], FP8, isOutput=False)  # * SW_F2
    t["b_ffn2_b"] = dp("b_ffn2_b", [P, D], F32, isOutput=False)
    t["out"] = dp("out", [LQ, D], F32, isOutput=True)
    return t


def _build_body(nc, tc, ctx, t):
    mm = nc.tensor.matmul
    dma = nc.sync.dma_start
    dmat = nc.sync.dma_start_transpose
    v = nc.vector
    gp = nc.gpsimd
    act = nc.scalar.activation
    RG = [list(range(NCORES))]

    const = ctx.enter_context(tc.tile_pool(name="const", bufs=1))
    eps_t = const.tile([P, 1], F32)
    v.memset(eps_t, EPS)

    dram = ctx.enter_context(tc.tile_pool(name="dram", bufs=1, space="DRAM"))
    ccA_in = dram.tile([ASL], F32)
    ccA_out = nc.dram_tensor(
        "ccA_out", [NCORES * ASL], F32, kind="Internal", addr_space="Shared"
    )
    ccKV_in = dram.tile([KBYTES + VBYTES], I8)
    ccKV_out = nc.dram_tensor(
        "ccKV_out", [NCORES, KBYTES + VBYTES], I8, kind="Internal",
        addr_space="Shared",
    )

    # persistent adaln results
    adaln = ctx.enter_context(tc.tile_pool(name="adaln", bufs=1))
    sh1_col = adaln.tile([P, NDC], F32)
    sp1_col = adaln.tile([P, NDC], F32)
    sh2_col = adaln.tile([P, NDC], F32)
    sp2_col = adaln.tile([P, NDC], F32)
    g1_row = adaln.tile([1, D], F32)
    g2_row = adaln.tile([1, D], F32)
    g1s_b = adaln.tile([P, D], F32)
    g2s_b = adaln.tile([P, D], F32)
    xb_bias = adaln.tile([P, D], F32)  # g1 * b_attn_eff
    x2b_bias = adaln.tile([P, D], F32)  # g2 * b_ffn2

    # ---------------- phase A: adaln partial projection + AllGather -------
    with ExitStack() as phA:
        pool = phA.enter_context(tc.tile_pool(name="phA", bufs=1))
        psA = phA.enter_context(tc.tile_pool(name="psA", bufs=2, space="PSUM"))

        cond_sb = pool.tile([P, NDC], F32)
        dma(out=cond_sb[:], in_=t["cond_t"][:])
        sc_f = pool.tile([P, NDC], F32)
        act(sc_f[:], cond_sb[:], AF.Silu)
        sc_bf = pool.tile([P, NDC], BF16)
        v.tensor_copy(sc_bf[:], sc_f[:])

        wad = pool.tile([P, NDC, ASL], BF16)
        dma(out=wad[:], in_=t["wad_sl"][:])
        bad = pool.tile([P, 5], F32)
        dma(out=bad[:], in_=t["bad_sl"][:])

        acol = pool.tile([P, 5], F32)
        for m in range(5):
            n = P if m < 4 else ASL - 4 * P
            ps = psA.tile([P, 1], F32)
            for dc in range(NDC):
                mm(
                    ps[0:n, :],
                    wad[:, dc, m * P : m * P + n],
                    sc_bf[:, dc : dc + 1],
                    start=(dc == 0),
                    stop=(dc == NDC - 1),
                )
            v.tensor_add(acol[0:n, m : m + 1], ps[0:n, :], bad[0:n, m : m + 1])
        dma(
            out=ccA_in[0 : 4 * P].rearrange("(m p) -> p m", p=P),
            in_=acol[:, 0:4],
        )
        dma(
            out=ccA_in[4 * P : ASL].rearrange("(m p) -> p m", p=ASL - 4 * P),
            in_=acol[0 : ASL - 4 * P, 4:5],
        )
        gp.collective_compute(
            "AllGather",
            OP.bypass,
            replica_groups=RG,
            ins=[ccA_in.opt()],
            outs=[ccA_out[:].opt()],
        )
        # gathered layout: flat col index of [w_adaln1 | w_adaln2]
        acc = ccA_out
        dma(out=sh1_col[:], in_=acc[0:D].rearrange("(c p) -> p c", p=P))
        sp1_raw = pool.tile([P, NDC], F32)
        dma(out=sp1_raw[:], in_=acc[D : 2 * D].rearrange("(c p) -> p c", p=P))
        v.tensor_scalar_add(sp1_col[:], sp1_raw[:], 1.0)
        dma(out=g1_row[:], in_=acc[2 * D : 3 * D].rearrange("(o d) -> o d", o=1))
        dma(out=sh2_col[:], in_=acc[3 * D : 4 * D].rearrange("(c p) -> p c", p=P))
        sp2_raw = pool.tile([P, NDC], F32)
        dma(out=sp2_raw[:], in_=acc[4 * D : 5 * D].rearrange("(c p) -> p c", p=P))
        v.tensor_scalar_add(sp2_col[:], sp2_raw[:], 1.0)
        dma(out=g2_row[:], in_=acc[5 * D : 6 * D].rearrange("(o d) -> o d", o=1))

    # ---------------- phase B: local LN1 + QKV + AllGather K/V ------------
    big = ctx.enter_context(tc.tile_pool(name="big", bufs=1))
    x_loc = big.tile([P, NQC, D], F32)
    x2_loc = [big.tile([P, D], F32, name=f"x2_loc{q}") for q in range(NQC)]
    catT8 = big.tile([P, NDC, LQ], FP8)
    xn2T8 = big.tile([P, NDC, LQ], FP8)

    s_attn = ctx.enter_context(ExitStack())
    attn_pool = s_attn.enter_context(tc.tile_pool(name="attn", bufs=1))
    kT_all = attn_pool.tile([P, NHP, L], BF16)
    v4 = attn_pool.tile([P, NKC, H * VE], FP8)
    qT_all = attn_pool.tile([P, NHP, LQ], BF16)

    dma(out=x_loc[:], in_=t["x"].rearrange("(n p) d -> p n d", p=P))
    v4r = v4.rearrange("p k (h e) -> p k h e", e=VE)
    v.memset(v4r[:, :, :, HD : HD + 1], 1.0 / SW_CAT)
    v.memset(v4r[:, :, :, HD + 1 :], 0.0)

    with ExitStack() as phB:
        qkv_pool = phB.enter_context(tc.tile_pool(name="qkv", bufs=1))
        spool = phB.enter_context(tc.tile_pool(name="spool", bufs=8))
        tpool = phB.enter_context(tc.tile_pool(name="tpool", bufs=1))
        psB1 = phB.enter_context(tc.tile_pool(name="psB1", bufs=2, space="PSUM"))
        psB2 = phB.enter_context(tc.tile_pool(name="psB2", bufs=2, space="PSUM"))

        w8qkv = qkv_pool.tile([P, NDC, 3 * D], FP8)
        dma(out=w8qkv[:], in_=t["w_qkv8"].rearrange("(c p) m -> p c m", p=P))
        bq_col = qkv_pool.tile([P, NDC], F32)
        dma(out=bq_col[:], in_=t["b_q_col"][:])
        xn1T8 = qkv_pool.tile([P, NDC, LQ], FP8)
        xn1T_bf = tpool.tile([P, NDC, LQ], BF16)

        # LN1 over the local 512 rows (fp32 stats)
        nxs = []
        for qc in range(NQC):
            stats = spool.tile([P, 2, 6], F32, tag="st")
            for g in range(2):
                v.bn_stats(stats[:, g, :], x_loc[:, qc, g * 384 : (g + 1) * 384])
            mv = spool.tile([P, 2], F32, tag="mv", name=f"mv{qc}")
            v.bn_aggr(mv[:], stats[:])
            sq = spool.tile([P, 1], F32, tag="sq")
            act(sq[:], mv[:, 1:2], AF.Sqrt, bias=eps_t[:, 0:1])
            rstd = spool.tile([P, 1], F32, tag="rstd", name=f"rstd{qc}")
            v.reciprocal_approx_fast(rstd[:], sq[:])
            nx = spool.tile([P, D], BF16, tag="nx", name=f"nx{qc}")
            v.tensor_scalar(
                nx[:], x_loc[:, qc, :], mv[:, 0:1], rstd[:, 0:1],
                op0=OP.subtract, op1=OP.mult,
            )
            nxs.append(nx)
        for qc in range(NQC):
            dmat(out=xn1T_bf[:, :, qc * P : (qc + 1) * P], in_=nxs[qc][:])
        for dc in range(NDC):
            v.tensor_scalar(
                xn1T8[:, dc, :], xn1T_bf[:, dc, :],
                sp1_col[:, dc : dc + 1], sh1_col[:, dc : dc + 1],
                op0=OP.mult, op1=OP.add,
            )

        # K projection (local rows) -> bf16, bias dropped (softmax-invariant)
        k_sb = qkv_pool.tile([P, NHP, LQ], BF16)
        for hp in range(NHP):
            ps_k = psB1.tile([P, LQ], F32)
            for dc2 in range(NDC // 2):
                mm(
                    ps_k[:],
                    w8qkv[:, 2 * dc2 : 2 * dc2 + 2, D + hp * P : D + (hp + 1) * P],
                    xn1T8[:, 2 * dc2 : 2 * dc2 + 2, :],
                    start=(dc2 == 0),
                    stop=(dc2 == NDC // 2 - 1),
                    perf_mode=DR,
                )
            act(k_sb[:, hp, :], ps_k[:], AF.Copy, scale=1.0 / SW_QKV)
        dma(
            out=ccKV_in[0:KBYTES].bitcast(BF16).rearrange(
                "(hp p s) -> p hp s", hp=NHP, p=P
            ),
            in_=k_sb[:],
        )

        # V projection (local rows) -> fp8, [seq, dim] orientation
        v_sb = qkv_pool.tile([P, NQC, D], FP8)
        for lc in range(NQC):
            ps_v = psB2.tile([P, D], F32)
            for dc2 in range(NDC // 2):
                lhs = xn1T8[:, 2 * dc2 : 2 * dc2 + 2, lc * P : (lc + 1) * P]
                mm(ps_v[:, 0:512], lhs,
                   w8qkv[:, 2 * dc2 : 2 * dc2 + 2, 2 * D : 2 * D + 512],
                   start=(dc2 == 0), stop=(dc2 == NDC // 2 - 1), perf_mode=DR)
                mm(ps_v[:, 512:D], lhs,
                   w8qkv[:, 2 * dc2 : 2 * dc2 + 2, 2 * D + 512 : 3 * D],
                   start=(dc2 == 0), stop=(dc2 == NDC // 2 - 1), perf_mode=DR)
            act(v_sb[:, lc, :], ps_v[:], AF.Copy, scale=1.0 / SW_QKV)
        dma(
            out=ccKV_in[KBYTES:].bitcast(FP8).rearrange(
                "(lc p m) -> p lc m", lc=NQC, p=P
            ),
            in_=v_sb[:],
        )
        gp.collective_compute(
            "AllGather",
            OP.bypass,
            replica_groups=RG,
            ins=[ccKV_in.opt()],
            outs=[ccKV_out[:].opt()],
        )

        # Q projection (local rows only) while the AllGather runs
        for hp in range(NHP):
            ps_q = psB1.tile([P, LQ], F32)
            for dc2 in range(NDC // 2):
                mm(
                    ps_q[:],
                    w8qkv[:, 2 * dc2 : 2 * dc2 + 2, hp * P : (hp + 1) * P],
                    xn1T8[:, 2 * dc2 : 2 * dc2 + 2, :],
                    start=(dc2 == 0),
                    stop=(dc2 == NDC // 2 - 1),
                    perf_mode=DR,
                )
            v.tensor_scalar(
                qT_all[:, hp, :], ps_q[:], 1.0 / SW_QKV, bq_col[:, hp : hp + 1],
                op0=OP.mult, op1=OP.add,
            )

        # gate broadcasts + residual bias folds (also inside the AG window)
        gp.partition_broadcast(g1s_b[:], g1_row[:])
        gp.partition_broadcast(g2s_b[:], g2_row[:])
        ba_sb = qkv_pool.tile([P, D], F32)
        dma(out=ba_sb[:], in_=t["b_attn_b"][:])
        bf2_sb = qkv_pool.tile([P, D], F32)
        dma(out=bf2_sb[:], in_=t["b_ffn2_b"][:])
        v.tensor_tensor(xb_bias[:], ba_sb[:], g1s_b[:], op=OP.mult)
        v.tensor_tensor(x2b_bias[:], bf2_sb[:], g2s_b[:], op=OP.mult)

        # gather-back: K into kT_all, V into v4 (layout change is free)
        kv_bf = ccKV_out.bitcast(BF16)  # [NCORES, (KBYTES+VBYTES)//2]
        for hp in range(NHP):
            dma(
                out=kT_all[:, hp, :].rearrange("p (r s) -> p r s", r=NCORES),
                in_=kv_bf[:, hp * (P * LQ) : (hp + 1) * (P * LQ)].rearrange(
                    "r (p s) -> p r s", p=P
                ),
            )
        kv_f8 = ccKV_out.bitcast(FP8)  # [NCORES, KBYTES+VBYTES]
        for r in range(NCORES):
            for lc in range(NQC):
                o = KBYTES + lc * (P * D)
                dma(
                    out=v4r[:, r * NQC + lc, :, 0:HD],
                    in_=kv_f8[r, o : o + P * D].rearrange(
                        "(p h d) -> p h d", p=P, h=H
                    ),
                )

    # ---------------- phase C: attention --------------------------------
    with ExitStack() as phC:
        pt_pool = phC.enter_context(tc.tile_pool(name="ptp", bufs=6))
        tiny = phC.enter_context(tc.tile_pool(name="tiny", bufs=4))
        psS = phC.enter_context(tc.tile_pool(name="psS", bufs=3, space="PSUM"))
        psO = phC.enter_context(tc.tile_pool(name="psO", bufs=1, space="PSUM"))

        NK2 = NKC // 2
        for hp in range(NHP):
            ps_o = [psO.tile([VE, LQ], F32, tag=f"ps_o{dlt}", name=f"psO{hp}_{dlt}") for dlt in range(2)]
            pending = None
            for kc2 in range(NK2):
                ps_s = [psS.tile([P, 1024], F32, tag="ps_s", name=f"ps_s{_d}") for _d in range(2)]
                for j in range(2):
                    kc = 2 * kc2 + j
                    for dlt in range(2):
                        off = dlt * HD
                        mm(
                            ps_s[dlt][:, j * 512 : (j + 1) * 512],
                            kT_all[off : off + HD, hp, kc * P : (kc + 1) * P],
                            qT_all[off : off + HD, hp, :],
                            start=True,
                            stop=True,
                        )
                pts = []
                for dlt in range(2):
                    ptile = pt_pool.tile([P, 1024], FP8E5, tag="pt", name=f"pt{dlt}")
                    if dlt == 0:
                        act(ptile[:], ps_s[dlt][:], AF.Exp, scale=0.125)
                    else:
                        v.tensor_scalar(
                            ptile.bitcast(I8)[:], ps_s[dlt][:], SCH_A, SCH_B,
                            op0=OP.mult, op1=OP.add,
                        )
                    pts.append(ptile)
                if pending is not None:
                    pk2, ppts = pending
                    for dlt in range(2):
                        h = 2 * hp + dlt
                        mm(
                            ps_o[dlt][:],
                            v4[:, 2 * pk2 : 2 * pk2 + 2, h * VE : (h + 1) * VE],
                            ppts[dlt].rearrange("p (j n) -> p j n", j=2)[:],
                            start=(pk2 == 0),
                            stop=False,
                            perf_mode=DR,
                        )
                pending = (kc2, pts)
            pk2, ppts = pending
            for dlt in range(2):
                h = 2 * hp + dlt
                mm(
                    ps_o[dlt][:],
                    v4[:, 2 * pk2 : 2 * pk2 + 2, h * VE : (h + 1) * VE],
                    ppts[dlt].rearrange("p (j n) -> p j n", j=2)[:],
                    start=False,
                    stop=True,
                    perf_mode=DR,
                )
            # normalize: cat = ps_o[0:64] * broadcast(1/z); z = ones row
            for dlt in range(2):
                off = dlt * HD
                zr = tiny.tile([1, LQ], F32, tag="zr", name=f"zr{dlt}")
                v.tensor_copy(zr[:], ps_o[dlt][HD : HD + 1, :])
                rz_f = tiny.tile([1, LQ], F32, tag="rz_f", name=f"rz_f{dlt}")
                v.reciprocal_approx_fast(rz_f[:], zr[:])
                rzb = tiny.tile([P, LQ], F32, tag="rzb", name=f"rzb{dlt}")
                gp.partition_broadcast(rzb[:], rz_f[:])
                v.tensor_tensor(
                    catT8[off : off + HD, hp, :],
                    ps_o[dlt][0:HD, :],
                    rzb[0:HD, :],
                    op=OP.mult,
                )

    s_attn.close()  # free K/V/Q before the FFN weights land

    # -------- phase D: out-projection, residual, LN2 ---------------------
    with ExitStack() as phD:
        pool = phD.enter_context(tc.tile_pool(name="phD", bufs=2))
        spool = phD.enter_context(tc.tile_pool(name="spoolE", bufs=4))
        tpool2 = phD.enter_context(tc.tile_pool(name="tpool2", bufs=1))
        psD1 = phD.enter_context(tc.tile_pool(name="psD1", bufs=2, space="PSUM"))
        psD2 = phD.enter_context(tc.tile_pool(name="psD2", bufs=2, space="PSUM"))

        w8ao = pool.tile([P, NDC, D], FP8, name="w8ao")
        dma(out=w8ao[:], in_=t["w_ao8"].rearrange("(c p) m -> p c m", p=P))
        # xbl = x + g1*b_attn_eff, the per-qc residual base
        xbl = [pool.tile([P, D], F32, name=f"xbl{q}") for q in range(NQC)]
        for q in range(NQC):
            v.tensor_add(xbl[q][:], x_loc[:, q, :], xb_bias[:])

        xn2T_bf = tpool2.tile([P, NDC, LQ], BF16)
        for qc in range(NQC):
            ps1 = psD1.tile([P, 512], F32)
            ps2 = psD2.tile([P, 256], F32)
            for cc2 in range(NDC // 2):
                lhs = catT8[:, 2 * cc2 : 2 * cc2 + 2, qc * P : (qc + 1) * P]
                mm(ps1[:], lhs, w8ao[:, 2 * cc2 : 2 * cc2 + 2, 0:512],
                   start=(cc2 == 0), stop=(cc2 == NDC // 2 - 1), perf_mode=DR)
                mm(ps2[:], lhs, w8ao[:, 2 * cc2 : 2 * cc2 + 2, 512:D],
                   start=(cc2 == 0), stop=(cc2 == NDC // 2 - 1), perf_mode=DR)
            # x2 = x + g1*(psum / (SW_AO*SW_CAT)) + g1*b
            gt = pool.tile([P, D], F32, tag="gt", name=f"gt{qc}")
            v.scalar_tensor_tensor(
                gt[:, 0:512], ps1[:], 1.0 / (SW_AO * SW_CAT), g1s_b[:, 0:512],
                op0=OP.mult, op1=OP.mult,
            )
            v.scalar_tensor_tensor(
                gt[:, 512:D], ps2[:], 1.0 / (SW_AO * SW_CAT), g1s_b[:, 512:D],
                op0=OP.mult, op1=OP.mult,
            )
            v.tensor_add(x2_loc[qc][:], gt[:], xbl[qc][:])
        for qc in range(NQC):
            stats = spool.tile([P, 2, 6], F32, tag="st2")
            for g in range(2):
                v.bn_stats(stats[:, g, :], x2_loc[qc][:, g * 384 : (g + 1) * 384])
            mv = spool.tile([P, 2], F32, tag="mv2", name=f"mv2_{qc}")
            v.bn_aggr(mv[:], stats[:])
            sq = spool.tile([P, 1], F32, tag="sq2")
            act(sq[:], mv[:, 1:2], AF.Sqrt, bias=eps_t[:, 0:1])
            rstd = spool.tile([P, 1], F32, tag="rstd2", name=f"rstd2_{qc}")
            v.reciprocal_approx_fast(rstd[:], sq[:])
            nx = spool.tile([P, D], BF16, tag="nx2", name=f"nx2_{qc}")
            v.tensor_scalar(
                nx[:], x2_loc[qc][:], mv[:, 0:1], rstd[:, 0:1],
                op0=OP.subtract, op1=OP.mult,
            )
            dmat(out=xn2T_bf[:, :, qc * P : (qc + 1) * P], in_=nx[:])
        for dc in range(NDC):
            v.tensor_scalar(
                xn2T8[:, dc, :], xn2T_bf[:, dc, :],
                sp2_col[:, dc : dc + 1], sh2_col[:, dc : dc + 1],
                op0=OP.mult, op1=OP.add,
            )

    # ---------------- phase F: FFN + gate + residual -> output -----------
    with ExitStack() as phF:
        wpool = phF.enter_context(tc.tile_pool(name="wffn", bufs=1))
        hpool = phF.enter_context(tc.tile_pool(name="hT", bufs=1))
        pool = phF.enter_context(tc.tile_pool(name="phF", bufs=2))
        psF1 = phF.enter_context(tc.tile_pool(name="psF1", bufs=3, space="PSUM"))
        psF2 = phF.enter_context(tc.tile_pool(name="psF2", bufs=2, space="PSUM"))

        w8f1 = wpool.tile([P, NDC, DM], FP8)
        dma(out=w8f1[:], in_=t["w_ffn18"].rearrange("(c p) m -> p c m", p=P))
        bf1_col = wpool.tile([P, NMC], F32)
        dma(out=bf1_col[:], in_=t["b_ffn1_col"][:])
        w8f2 = wpool.tile([P, NMC, D], FP8)
        dma(out=w8f2[:], in_=t["w_f28"].rearrange("(c p) m -> p c m", p=P))
        # x2 with the gated ffn2 bias folded in (per-qc residual base)
        for q in range(NQC):
            v.tensor_add(x2_loc[q][:], x2_loc[q][:], x2b_bias[:])

        hT8 = hpool.tile([P, NMC, LQ], FP8)
        for mc in range(NMC):
            ps_h = psF1.tile([P, 512], F32, tag="mm512")
            for dc2 in range(NDC // 2):
                mm(
                    ps_h[:],
                    w8f1[:, 2 * dc2 : 2 * dc2 + 2, mc * P : (mc + 1) * P],
                    xn2T8[:, 2 * dc2 : 2 * dc2 + 2, :],
                    start=(dc2 == 0),
                    stop=(dc2 == NDC // 2 - 1),
                    perf_mode=DR,
                )
            act(
                hT8[:, mc, :], ps_h[:], AF.Gelu,
                bias=bf1_col[:, mc : mc + 1], scale=1.0 / SW_F1,
            )

        out_r = t["out"].rearrange("(n p) d -> n p d", p=P)
        for qc in range(NQC):
            ps1 = psF1.tile([P, 512], F32, tag="mm512")
            ps2 = psF2.tile([P, 256], F32)
            for mc2 in range(NMC // 2):
                lhs = hT8[:, 2 * mc2 : 2 * mc2 + 2, qc * P : (qc + 1) * P]
                mm(ps1[:], lhs, w8f2[:, 2 * mc2 : 2 * mc2 + 2, 0:512],
                   start=(mc2 == 0), stop=(mc2 == NMC // 2 - 1), perf_mode=DR)
                mm(ps2[:], lhs, w8f2[:, 2 * mc2 : 2 * mc2 + 2, 512:D],
                   start=(mc2 == 0), stop=(mc2 == NMC // 2 - 1), perf_mode=DR)
            gt = pool.tile([P, D], F32, tag="gt")
            v.scalar_tensor_tensor(
                gt[:, 0:512], ps1[:], 1.0 / SW_F2, g2s_b[:, 0:512],
                op0=OP.mult, op1=OP.mult,
            )
            v.scalar_tensor_tensor(
                gt[:, 512:D], ps2[:], 1.0 / SW_F2, g2s_b[:, 512:D],
                op0=OP.mult, op1=OP.mult,
            )
            ot = pool.tile([P, D], F32)
            v.tensor_add(ot[:], gt[:], x2_loc[qc][:])
            dma(out=out_r[qc], in_=ot[:])


def build_nc():
    nc = bacc.Bacc(
        None, target_bir_lowering=False, debug=False, num_devices=NCORES
    )
    t = _declare_params(nc)
    with tile.TileContext(nc) as tc:
        with ExitStack() as ctx:
            _build_body(nc, tc, ctx, t)
    nc.compile()
    return nc


_cache = {}


def _prep_in_maps(inputs):
    E4 = ml_dtypes.float8_e4m3fn
    bf = lambda a: np.ascontiguousarray(np.asarray(a, np.float32)).astype(
        ml_dtypes.bfloat16
    )
    f32 = lambda a: np.ascontiguousarray(np.asarray(a, np.float32))
    q8 = lambda a, s: np.ascontiguousarray(
        (np.asarray(a, np.float32) * s).astype(E4)
    )
    x = f32(inputs["x"]).reshape(L, D)
    cond = f32(inputs["cond"]).reshape(D)
    b_qkv = f32(inputs["b_qkv"]).reshape(3 * D)
    w_ao = f32(inputs["w_attn_out"])
    # fold the V bias through the out-projection: b_attn_eff = b + bv @ Wao
    b_attn_eff = f32(inputs["b_attn_out"]).reshape(D) + b_qkv[2 * D :] @ w_ao
    w_ad = np.concatenate(
        [f32(inputs["w_adaln1"]), f32(inputs["w_adaln2"])], axis=1
    )  # [D, 4608]
    b_ad = np.concatenate(
        [f32(inputs["b_adaln1"]).reshape(-1), f32(inputs["b_adaln2"]).reshape(-1)]
    )  # [4608]
    common = {
        "cond_t": np.ascontiguousarray(cond.reshape(NDC, P).T),
        "w_qkv8": q8(inputs["w_qkv"], SW_QKV),
        "b_q_col": np.ascontiguousarray(b_qkv[:D].reshape(NDC, P).T),
        "w_ao8": q8(w_ao, SW_AO),
        "b_attn_b": np.ascontiguousarray(np.broadcast_to(b_attn_eff, (P, D))),
        "w_ffn18": q8(inputs["w_ffn1"], SW_F1),
        "b_ffn1_col": np.ascontiguousarray(
            f32(inputs["b_ffn1"]).reshape(NMC, P).T
        ),
        "w_f28": q8(inputs["w_ffn2"], SW_F2),
        "b_ffn2_b": np.ascontiguousarray(
            np.broadcast_to(f32(inputs["b_ffn2"]).reshape(D), (P, D))
        ),
    }
    in_maps = []
    for c in range(NCORES):
        m = dict(common)
        xr = np.roll(x, -c * LQ, axis=0)
        m["x"] = np.ascontiguousarray(xr[:LQ])
        wsl = w_ad[:, c * ASL : (c + 1) * ASL]  # [768, 576]
        m["wad_sl"] = np.ascontiguousarray(
            wsl.reshape(NDC, P, ASL).transpose(1, 0, 2)
        ).astype(ml_dtypes.bfloat16)
        bsl = b_ad[c * ASL : (c + 1) * ASL]
        bcol = np.zeros((P, 5), np.float32)
        bcol[:, 0:4] = bsl[: 4 * P].reshape(4, P).T
        bcol[: ASL - 4 * P, 4] = bsl[4 * P :]
        m["bad_sl"] = np.ascontiguousarray(bcol)
        in_maps.append(m)
    return in_maps


def kernel(**inputs):
    if "nc" not in _cache:
        _cache["nc"] = build_nc()
    nc = _cache["nc"]
    in_maps = _prep_in_maps(inputs)
    res = run_bass_kernel_spmd(nc, in_maps, list(range(NCORES)))
    out = np.concatenate([res.results[c]["out"] for c in range(NCORES)], axis=0)
    return out.reshape(1, L, D).astype(np.float32)


if __name__ == "__main__":
    rng = np.random.default_rng(0)
    fake = {
        "x": rng.standard_normal((1, L, D), dtype=np.float32),
        "cond": rng.standard_normal((1, D), dtype=np.float32),
        "w_adaln1": rng.standard_normal((D, 3 * D), dtype=np.float32) * 0.02,
        "b_adaln1": np.zeros(3 * D, np.float32),
        "w_qkv": rng.standard_normal((D, 3 * D), dtype=np.float32) * D**-0.5,
        "b_qkv": np.zeros(3 * D, np.float32),
        "w_attn_out": rng.standard_normal((D, D), dtype=np.float32) * D**-0.5,
        "b_attn_out": np.zeros(D, np.float32),
        "w_adaln2": rng.standard_normal((D, 3 * D), dtype=np.float32) * 0.02,
        "b_adaln2": np.zeros(3 * D, np.float32),
        "w_ffn1": rng.standard_normal((D, DM), dtype=np.float32) * D**-0.5,
        "b_ffn1": np.zeros(DM, np.float32),
        "w_ffn2": rng.standard_normal((DM, D), dtype=np.float32) * DM**-0.5,
        "b_ffn2": np.zeros(D, np.float32),
    }
    out = kernel(**fake)
    print("out", out.shape, out.dtype, np.abs(out).max())
